# revision 4
# baseline (speedup 1.0000x reference)
"""Causal multi-head attention (B=2, S=2048, D=2048, 32 heads x 64) for 8
Trainium2 NeuronCores.

Sharding: data parallel on batch (2 groups of 4 cores) x tensor parallel on
heads (4 groups of 8 heads). Each core computes q/k/v projections for its
head group, RoPE, causal attention with sigmoid-gated values, and a partial
o-projection; the host sums the 4 partials per batch (the "all-reduce") and
adds the output bias.

All matmuls run in float32r (full PE rate, ~1e-4 relative precision).
"""

import os

import numpy as np

import concourse.bacc as bacc
import concourse.tile as tile
from concourse import mybir
from concourse.bass_utils import run_bass_kernel_spmd

B, S, D = 2, 2048, 2048
H_PER_CORE = 8          # heads per core
DH = 64                 # head dim
CW = 512                # per-core projection width = H_PER_CORE * DH
N_CORES = 8
KT = D // 128           # k-subtiles for the D-contraction

f32 = mybir.dt.float32
f32r = mybir.dt.float32r
Act = mybir.ActivationFunctionType

TRACE = bool(int(os.environ.get("KERNEL_TRACE", "0")))
LAST_EXEC_NS = None


def _build():
    nc = bacc.Bacc("TRN2", target_bir_lowering=False, debug=False)

    xT = nc.dram_tensor("xT", [D, S], f32, kind="ExternalInput")
    wq = nc.dram_tensor("wq", [D, CW], f32, kind="ExternalInput")
    wk = nc.dram_tensor("wk", [D, CW], f32, kind="ExternalInput")
    wv = nc.dram_tensor("wv", [D, CW], f32, kind="ExternalInput")
    wo = nc.dram_tensor("wo", [CW, D], f32, kind="ExternalInput")
    bq = nc.dram_tensor("bq", [128, 4], f32, kind="ExternalInput")
    bk = nc.dram_tensor("bk", [128, 4], f32, kind="ExternalInput")
    bv = nc.dram_tensor("bv", [1, CW], f32, kind="ExternalInput")
    cosT = nc.dram_tensor("cosT", [128, S], f32, kind="ExternalInput")
    sinT = nc.dram_tensor("sinT", [128, S], f32, kind="ExternalInput")
    masks = nc.dram_tensor("masks", [128, 4, 512], f32, kind="ExternalInput")
    vones = nc.dram_tensor("vones", [128, 520], f32, kind="ExternalInput")
    part = nc.dram_tensor("part", [S, D], f32, kind="ExternalOutput")

    x3 = xT[:].rearrange("(kt p) s -> p kt s", p=128)      # [128, 16, S]
    wq3 = wq[:].rearrange("(kt p) m -> p kt m", p=128)     # [128, 16, 512]
    wk3 = wk[:].rearrange("(kt p) m -> p kt m", p=128)
    wv3 = wv[:].rearrange("(kt p) m -> p kt m", p=128)
    wo3 = wo[:].rearrange("(kc p) m -> p kc m", p=128)     # [128, 4, 2048]

    with tile.TileContext(nc) as tc:
        with (
            tc.tile_pool(name="p0", bufs=1) as p0,
            tc.tile_pool(name="pqk", bufs=1) as pqk,
        ):
            # persistent state
            qt = [pqk.tile([128, S], f32r, name=f"qt{i}") for i in range(4)]
            kt = [pqk.tile([128, S], f32r, name=f"kt{i}") for i in range(4)]
            # one flat tile: 16 x [128, 520] v-slabs, then consts packed in
            # (SBUF tiles pad to 4KB/partition, so small tiles are wasteful)
            va_all = p0.tile([128, 16 * 520 + 8 + 128 + CW], f32r, name="va_all")
            va = [va_all[:, 520 * i:520 * (i + 1)] for i in range(16)]
            bqt = va_all[:, 8320:8324]
            bkt = va_all[:, 8324:8328]
            ones = va_all[0:1, 8328:8456]
            bvt = va_all[0:1, 8456:8456 + CW]

            nc.sync.dma_start(ones, vones[0:1, 0:128].bitcast(f32r))
            nc.sync.dma_start(bvt, bv[:].bitcast(f32r))
            nc.sync.dma_start(bqt, bq[:].bitcast(f32r))
            nc.sync.dma_start(bkt, bk[:].bitcast(f32r))
            # va default 1.0 -> the per-head 65th column stays 1 (ones column
            # for the softmax denominator); data columns are overwritten.
            for i in range(16):
                nc.sync.dma_start(va[i], vones[:].bitcast(f32r))

            # ---------------- Phase A: projections ----------------
            for half in range(2):
                with tc.tile_pool(name="pa", bufs=1) as pa:
                    xth = pa.tile([128, KT, 1024], f32r, name="xth")
                    for k in range(KT):
                        nc.sync.dma_start(
                            xth[:, k, :],
                            x3[:, k, half * 1024:(half + 1) * 1024].bitcast(f32r),
                        )
                    # Q and K projections (M-chunked weights)
                    with (
                        tc.tile_pool(name="paw", bufs=3) as paw,
                        tc.tile_pool(name="psa", bufs=4, space="PSUM") as psa,
                    ):
                        for w3, dst, bias, scale in (
                            (wq3, qt, bqt, 1.0),
                            (wk3, kt, bkt, 0.125),
                        ):
                            for mt in range(4):
                                wch = paw.tile([128, KT, 128], f32r, tag="wch",
                                               name="wch")
                                nc.sync.dma_start(
                                    wch[:], w3[:, :, mt * 128:(mt + 1) * 128].bitcast(f32r)
                                )
                                for nt in range(2):
                                    ps = psa.tile([128, 512], f32, tag="psa",
                                                  name="ps_a")
                                    for k in range(KT):
                                        nc.tensor.matmul(
                                            ps[:], wch[:, k, :],
                                            xth[:, k, nt * 512:(nt + 1) * 512],
                                            start=(k == 0), stop=(k == KT - 1),
                                        )
                                    col = half * 1024 + nt * 512
                                    nc.scalar.activation(
                                        dst[mt][:, col:col + 512], ps[:],
                                        Act.Identity,
                                        bias=bias[:, mt:mt + 1].bitcast(f32), scale=scale,
                                    )
                    # V projection (full-width rhs, s-chunked lhsT)
                    with (
                        tc.tile_pool(name="pav", bufs=1) as pav,
                        tc.tile_pool(name="psv", bufs=2, space="PSUM") as psv,
                    ):
                        wvf = pav.tile([128, KT, CW], f32r, name="wvf")
                        for k in range(KT):
                            nc.sync.dma_start(wvf[:, k, :], wv3[:, k, :].bitcast(f32r))
                        for st in range(8):
                            stg = half * 8 + st
                            ps = psv.tile([128, CW], f32, tag="psv", name="ps_v")
                            for k in range(KT):
                                nc.tensor.matmul(
                                    ps[:], xth[:, k, st * 128:(st + 1) * 128],
                                    wvf[:, k, :],
                                    start=(k == 0), stop=False,
                                )
                            nc.tensor.matmul(
                                ps[:], ones, bvt,
                                start=False, stop=True,
                            )
                            for h in range(H_PER_CORE):
                                nc.scalar.activation(
                                    va[stg][:, 65 * h:65 * h + 64],
                                    ps[:, 64 * h:64 * h + 64],
                                    Act.Sigmoid,
                                )

            # ---------------- Phase A2: RoPE on Q and K ----------------
            with (
                tc.tile_pool(name="prc", bufs=1) as prc,
                tc.tile_pool(name="prt", bufs=2) as prt,
            ):
                cost = prc.tile([128, S], f32, name="cost")
                sint = prc.tile([128, S], f32, name="sint")
                nc.sync.dma_start(cost[:], cosT[:])
                nc.sync.dma_start(sint[:], sinT[:])
                for t in qt + kt:
                    rot = prt.tile([128, S], f32r, tag="rot", name="rot")
                    for base in (0, 64):
                        nc.vector.tensor_scalar_mul(
                            rot[base:base + 32, :], t[base + 32:base + 64, :], -1.0
                        )
                        nc.vector.tensor_copy(
                            rot[base + 32:base + 64, :], t[base:base + 32, :]
                        )
                    nc.vector.tensor_mul(rot[:], rot[:], sint[:])
                    nc.vector.tensor_mul(t[:], t[:], cost[:])
                    nc.vector.tensor_add(t[:], t[:], rot[:])

            # ---------------- Phase B: causal attention ----------------
            with tc.tile_pool(name="py", bufs=1) as py:
                ytr = [py.tile([128, S], f32r, name=f"ytr{i}") for i in range(4)]
                with (
                    tc.tile_pool(name="pb", bufs=1) as pb,
                    tc.tile_pool(name="pba", bufs=6) as pba,
                    tc.tile_pool(name="pbs", bufs=2) as pbs,
                    tc.tile_pool(name="pss", bufs=4, space="PSUM") as pss,
                    tc.tile_pool(name="psy", bufs=2, space="PSUM") as psy,
                ):
                    maskt = pb.tile([128, 4, 512], f32, name="maskt")
                    nc.sync.dma_start(maskt[:], masks[:])
                    kscr = [pb.tile([128, S], f32r, name=f"kscr{i}")
                            for i in range(2)]
                    # zero the pad halves (x0.0 of initialized data emits
                    # f32r-rounded zeros, which gpsimd memset cannot)
                    nc.vector.tensor_scalar_mul(kscr[0][64:128, :], qt[0][64:128, :], 0.0)
                    nc.vector.tensor_scalar_mul(kscr[1][0:64, :], qt[0][0:64, :], 0.0)

                    for pi in range(4):
                        for hh in range(2):
                            h = 2 * pi + hh
                            ks = kscr[hh]
                            lo, hi = hh * 64, (hh + 1) * 64
                            nc.vector.tensor_copy(
                                ks[lo:hi, :], kt[pi][lo:hi, :]
                            )
                            for qb in range(4):
                                nkt = 4 * qb + 4
                                yps = psy.tile([65, 512], f32, tag="yps",
                                               name="ps_y")
                                for k_i in range(nkt):
                                    ps = pss.tile([128, 512], f32, tag="pss",
                                                  name="ps_s")
                                    nc.tensor.matmul(
                                        ps[:],
                                        ks[:, k_i * 128:(k_i + 1) * 128],
                                        qt[pi][:, qb * 512:(qb + 1) * 512],
                                        start=True, stop=True,
                                    )
                                    dt_i = k_i - 4 * qb
                                    if dt_i >= 0:
                                        nc.vector.tensor_add(
                                            ps[:], ps[:], maskt[:, dt_i, :]
                                        )
                                    at = pba.tile([128, 512], f32r, tag="at",
                                                  name="at")
                                    nc.scalar.activation(at[:], ps[:], Act.Exp)
                                    nc.tensor.matmul(
                                        yps[:],
                                        va[k_i][:, 65 * h:65 * h + 65],
                                        at[:],
                                        start=(k_i == 0), stop=(k_i == nkt - 1),
                                    )
                                rc = pbs.tile([1, 512], f32, tag="rc", name="rc")
                                nc.vector.reciprocal(rc[:], yps[64:65, :])
                                s64 = pbs.tile([64, 512], f32, tag="s64",
                                               name="s64")
                                nc.gpsimd.partition_broadcast(s64[:], rc[:])
                                nc.vector.tensor_mul(
                                    ytr[pi][lo:hi, qb * 512:(qb + 1) * 512],
                                    yps[0:64, :], s64[:],
                                )

                # ---------------- Phase C: o-projection partial ----------------
                with (
                    tc.tile_pool(name="pc", bufs=3) as pc,
                    tc.tile_pool(name="pso", bufs=3, space="PSUM") as pso,
                ):
                    for nt in range(4):
                        woc = pc.tile([128, 4, 512], f32r, tag="woc", name="woc")
                        for kc in range(4):
                            nc.sync.dma_start(
                                woc[:, kc, :],
                                wo3[:, kc, nt * 512:(nt + 1) * 512].bitcast(f32r),
                            )
                        for st in range(16):
                            ps = pso.tile([128, 512], f32, tag="pso", name="ps_o")
                            for kc in range(4):
                                nc.tensor.matmul(
                                    ps[:],
                                    ytr[kc][:, st * 128:(st + 1) * 128],
                                    woc[:, kc, :],
                                    start=(kc == 0), stop=(kc == 3),
                                )
                            ostg = pc.tile([128, 512], f32, tag="ostg",
                                           name="ostg")
                            nc.scalar.copy(ostg[:], ps[:])
                            nc.sync.dma_start(
                                part[st * 128:(st + 1) * 128,
                                     nt * 512:(nt + 1) * 512],
                                ostg[:],
                            )

    nc.compile()
    return nc


def _rope_tables():
    half = DH // 2
    inv_freq = 1.0 / (10000.0 ** (np.arange(0, half, dtype=np.float32) / half))
    t = np.arange(S, dtype=np.float32)
    freqs = np.einsum("i,j->ij", t, inv_freq)            # [S, 32]
    emb = np.concatenate([freqs, freqs], axis=-1)        # [S, 64]
    cos = np.cos(emb).T.astype(np.float32)               # [64, S]
    sin = np.sin(emb).T.astype(np.float32)
    cosT = np.ascontiguousarray(np.tile(cos, (2, 1)))    # [128, S]
    sinT = np.ascontiguousarray(np.tile(sin, (2, 1)))
    return cosT, sinT


def _masks():
    j = np.arange(128)[:, None, None]
    dt = np.arange(4)[None, :, None]
    i = np.arange(512)[None, None, :]
    keep = (128 * dt + j) <= i
    return np.where(keep, 0.0, -1e30).astype(np.float32)  # [128, 4, 512]


def kernel(**inputs):
    global LAST_EXEC_NS
    x = np.asarray(inputs["x"], dtype=np.float32)
    Wq = np.asarray(inputs["Wq"], dtype=np.float32)
    Wk = np.asarray(inputs["Wk"], dtype=np.float32)
    Wv = np.asarray(inputs["Wv"], dtype=np.float32)
    Wo = np.asarray(inputs["Wo"], dtype=np.float32)
    bq = np.asarray(inputs["bq"], dtype=np.float32)
    bk = np.asarray(inputs["bk"], dtype=np.float32)
    bv = np.asarray(inputs["bv"], dtype=np.float32)
    bo = np.asarray(inputs["bo"], dtype=np.float32)

    cosT, sinT = _rope_tables()
    masks = _masks()

    nc = _build()
    in_maps = []
    for c in range(N_CORES):
        b, g = c // 4, c % 4
        sl = slice(CW * g, CW * (g + 1))
        in_maps.append({
            "xT": np.ascontiguousarray(x[b].T),
            "wq": np.ascontiguousarray(Wq[sl].T),
            "wk": np.ascontiguousarray(Wk[sl].T),
            "wv": np.ascontiguousarray(Wv[sl].T),
            "wo": np.ascontiguousarray(Wo[:, sl].T),
            "bq": np.ascontiguousarray(bq[sl].reshape(4, 128).T),
            "bk": np.ascontiguousarray((bk[sl] * 0.125).reshape(4, 128).T),
            "bv": np.ascontiguousarray(bv[sl].reshape(1, CW)),
            "cosT": cosT,
            "vones": np.ones((128, 520), dtype=np.float32),
            "sinT": sinT,
            "masks": masks,
        })

    kwargs = {}
    if TRACE:
        kwargs = dict(trace=True, trace_cores=list(range(N_CORES)),
                      stitch_traces=False)
    r = run_bass_kernel_spmd(nc, in_maps, list(range(N_CORES)), **kwargs)
    LAST_EXEC_NS = r.exec_time_ns

    out = np.empty((B, S, D), dtype=np.float32)
    for b in range(B):
        acc = r.results[4 * b]["part"].astype(np.float32).copy()
        for g in range(1, 4):
            acc += r.results[4 * b + g]["part"]
        out[b] = acc + bo
    return out


# revision 5
# speedup vs baseline: 1.0502x; 1.0502x over previous
"""Causal multi-head attention (B=2, S=2048, D=2048, 32 heads x 64) for 8
Trainium2 NeuronCores.

Sharding: data parallel on batch (2 groups of 4 cores) x tensor parallel on
heads (4 groups of 8 heads). Each core computes q/k/v projections for its
head group, RoPE, causal attention with sigmoid-gated values, and a partial
o-projection; the host sums the 4 partials per batch (the "all-reduce") and
adds the output bias.

All matmuls run in float32r (full PE rate, ~1e-4 relative precision).
"""

import os

import numpy as np

import concourse.bacc as bacc
import concourse.tile as tile
from concourse import mybir
from concourse.bass_utils import run_bass_kernel_spmd

B, S, D = 2, 2048, 2048
H_PER_CORE = 8          # heads per core
DH = 64                 # head dim
CW = 512                # per-core projection width = H_PER_CORE * DH
N_CORES = 8
KT = D // 128           # k-subtiles for the D-contraction

f32 = mybir.dt.float32
f32r = mybir.dt.float32r
Act = mybir.ActivationFunctionType

TRACE = bool(int(os.environ.get("KERNEL_TRACE", "0")))
LAST_EXEC_NS = None


def _build():
    nc = bacc.Bacc("TRN2", target_bir_lowering=False, debug=False)

    xT = nc.dram_tensor("xT", [D, S], f32, kind="ExternalInput")
    wq = nc.dram_tensor("wq", [D, CW], f32, kind="ExternalInput")
    wk = nc.dram_tensor("wk", [D, CW], f32, kind="ExternalInput")
    wv = nc.dram_tensor("wv", [D, CW], f32, kind="ExternalInput")
    wo = nc.dram_tensor("wo", [CW, D], f32, kind="ExternalInput")
    bq = nc.dram_tensor("bq", [1, CW], f32, kind="ExternalInput")
    bk = nc.dram_tensor("bk", [1, CW], f32, kind="ExternalInput")
    bv = nc.dram_tensor("bv", [1, CW], f32, kind="ExternalInput")
    cosT = nc.dram_tensor("cosT", [64, S], f32, kind="ExternalInput")
    rsinT = nc.dram_tensor("rsinT", [64, S], f32, kind="ExternalInput")
    masks = nc.dram_tensor("masks", [128, 4, 512], f32, kind="ExternalInput")
    vones = nc.dram_tensor("vones", [128, 520], f32, kind="ExternalInput")
    part = nc.dram_tensor("part", [S, D], f32, kind="ExternalOutput")

    x3 = xT[:].rearrange("(kt p) s -> p kt s", p=128)      # [128, 16, S]
    wq3 = wq[:].rearrange("(kt p) m -> p kt m", p=128)     # [128, 16, 512]
    wk3 = wk[:].rearrange("(kt p) m -> p kt m", p=128)
    wv3 = wv[:].rearrange("(kt p) m -> p kt m", p=128)
    wo3 = wo[:].rearrange("(kc p) m -> p kc m", p=128)     # [128, 4, 2048]

    with tile.TileContext(nc) as tc:
        with (
            tc.tile_pool(name="p0", bufs=1) as p0,
            tc.tile_pool(name="pqk", bufs=1) as pqk,
        ):
            # persistent state
            qt = [pqk.tile([128, S], f32r, name=f"qt{i}") for i in range(4)]
            kt = [pqk.tile([128, S], f32r, name=f"kt{i}") for i in range(4)]
            # one flat tile: 16 x [128, 520] v-slabs, then row-consts
            # (SBUF tiles pad to 4KB/partition, so small tiles are wasteful)
            va_all = p0.tile([128, 16 * 520 + 4 * CW], f32r, name="va_all")
            va = [va_all[:, 520 * i:520 * (i + 1)] for i in range(16)]
            ones = va_all[0:1, 8320:8320 + CW]
            bvt = va_all[0:1, 8832:8832 + CW]
            bqrow = va_all[0:1, 9344:9344 + CW]
            bkrow = va_all[0:1, 9856:9856 + CW]

            nc.sync.dma_start(ones, vones[0:1, 0:CW].bitcast(f32r))
            nc.sync.dma_start(bvt, bv[:].bitcast(f32r))
            nc.sync.dma_start(bqrow, bq[:].bitcast(f32r))
            nc.sync.dma_start(bkrow, bk[:].bitcast(f32r))
            # va default 1.0 -> the per-head 65th column stays 1 (ones column
            # for the softmax denominator); data columns are overwritten.
            for i in range(16):
                nc.sync.dma_start(va[i], vones[:].bitcast(f32r))

            # ---------------- Phase A: projections ----------------
            for half in range(2):
                with tc.tile_pool(name="pa", bufs=1) as pa:
                    xth = pa.tile([128, KT, 1024], f32r, name="xth")
                    for k in range(KT):
                        nc.sync.dma_start(
                            xth[:, k, :],
                            x3[:, k, half * 1024:(half + 1) * 1024].bitcast(f32r),
                        )
                    # Q and K projections (M-chunked weights), RoPE fused
                    # into the PSUM->SBUF move on the vector engine.
                    with (
                        tc.tile_pool(name="paw", bufs=2) as paw,
                        tc.tile_pool(name="prc", bufs=1) as prc,
                        tc.tile_pool(name="prt", bufs=2) as prt,
                        tc.tile_pool(name="psa", bufs=4, space="PSUM") as psa,
                    ):
                        cosc = prc.tile([64, S], f32, name="cosc")
                        rsin = prc.tile([64, S], f32, name="rsin")
                        nc.sync.dma_start(cosc[:], cosT[:])
                        nc.sync.dma_start(rsin[:], rsinT[:])
                        for w3, dst, brow in (
                            (wq3, qt, bqrow),
                            (wk3, kt, bkrow),
                        ):
                            for mt in range(4):
                                wch = paw.tile([128, KT, 128], f32r, tag="wch",
                                               name="wch")
                                nc.sync.dma_start(
                                    wch[:], w3[:, :, mt * 128:(mt + 1) * 128].bitcast(f32r)
                                )
                                for nt in range(2):
                                    ps = psa.tile([128, 512], f32, tag="psa",
                                                  name="ps_a")
                                    for k in range(KT):
                                        nc.tensor.matmul(
                                            ps[:], wch[:, k, :],
                                            xth[:, k, nt * 512:(nt + 1) * 512],
                                            start=(k == 0), stop=False,
                                        )
                                    nc.tensor.matmul(
                                        ps[:], brow[:, mt * 128:(mt + 1) * 128],
                                        ones, start=False, stop=True,
                                    )
                                    col = half * 1024 + nt * 512
                                    cs = slice(col, col + 512)
                                    d = dst[mt]
                                    # cos part (table rows shared by both heads)
                                    nc.vector.tensor_mul(
                                        d[0:64, cs], ps[0:64, :], cosc[:, cs])
                                    nc.vector.tensor_mul(
                                        d[64:128, cs], ps[64:128, :], cosc[:, cs])
                                    # rotate-half * sin (sign folded into rsin)
                                    tmp = prt.tile([128, 512], f32r, tag="tmp",
                                                   name="tmp")
                                    for b0 in (0, 64):
                                        nc.vector.tensor_mul(
                                            tmp[b0:b0 + 32, :],
                                            ps[b0 + 32:b0 + 64, :],
                                            rsin[0:32, cs])
                                        nc.vector.tensor_mul(
                                            tmp[b0 + 32:b0 + 64, :],
                                            ps[b0:b0 + 32, :],
                                            rsin[32:64, cs])
                                    nc.vector.tensor_add(
                                        d[:, cs], d[:, cs], tmp[:])
                    # V projection (full-width rhs, s-chunked lhsT)
                    with (
                        tc.tile_pool(name="pav", bufs=1) as pav,
                        tc.tile_pool(name="psv", bufs=2, space="PSUM") as psv,
                    ):
                        wvf = pav.tile([128, KT, CW], f32r, name="wvf")
                        for k in range(KT):
                            nc.sync.dma_start(wvf[:, k, :], wv3[:, k, :].bitcast(f32r))
                        for st in range(8):
                            stg = half * 8 + st
                            ps = psv.tile([128, CW], f32, tag="psv", name="ps_v")
                            for k in range(KT):
                                nc.tensor.matmul(
                                    ps[:], xth[:, k, st * 128:(st + 1) * 128],
                                    wvf[:, k, :],
                                    start=(k == 0), stop=False,
                                )
                            nc.tensor.matmul(
                                ps[:], ones[:, 0:128], bvt,
                                start=False, stop=True,
                            )
                            for h in range(H_PER_CORE):
                                nc.scalar.activation(
                                    va[stg][:, 65 * h:65 * h + 64],
                                    ps[:, 64 * h:64 * h + 64],
                                    Act.Sigmoid,
                                )

            # ---------------- Phase B: causal attention ----------------
            with tc.tile_pool(name="py", bufs=1) as py:
                ytr = [py.tile([128, S], f32r, name=f"ytr{i}") for i in range(4)]
                with (
                    tc.tile_pool(name="pb", bufs=1) as pb,
                    tc.tile_pool(name="pba", bufs=6) as pba,
                    tc.tile_pool(name="pbs", bufs=2) as pbs,
                    tc.tile_pool(name="pss", bufs=4, space="PSUM") as pss,
                    tc.tile_pool(name="psy", bufs=2, space="PSUM") as psy,
                ):
                    maskt = pb.tile([128, 4, 512], f32, name="maskt")
                    nc.sync.dma_start(maskt[:], masks[:])
                    kscr = [pb.tile([128, S], f32r, name=f"kscr{i}")
                            for i in range(2)]
                    # zero the pad halves (x0.0 of initialized data emits
                    # f32r-rounded zeros, which gpsimd memset cannot)
                    nc.vector.tensor_scalar_mul(kscr[0][64:128, :], qt[0][64:128, :], 0.0)
                    nc.vector.tensor_scalar_mul(kscr[1][0:64, :], qt[0][0:64, :], 0.0)

                    for pi in range(4):
                        for hh in range(2):
                            h = 2 * pi + hh
                            ks = kscr[hh]
                            lo, hi = hh * 64, (hh + 1) * 64
                            nc.vector.tensor_copy(
                                ks[lo:hi, :], kt[pi][lo:hi, :]
                            )
                            for qb in range(4):
                                nkt = 4 * qb + 4
                                yps = psy.tile([65, 512], f32, tag="yps",
                                               name="ps_y")
                                for k_i in range(nkt):
                                    ps = pss.tile([128, 512], f32, tag="pss",
                                                  name="ps_s")
                                    nc.tensor.matmul(
                                        ps[:],
                                        ks[:, k_i * 128:(k_i + 1) * 128],
                                        qt[pi][:, qb * 512:(qb + 1) * 512],
                                        start=True, stop=True,
                                    )
                                    dt_i = k_i - 4 * qb
                                    if dt_i >= 0:
                                        nc.vector.tensor_add(
                                            ps[:], ps[:], maskt[:, dt_i, :]
                                        )
                                    at = pba.tile([128, 512], f32r, tag="at",
                                                  name="at")
                                    nc.scalar.activation(at[:], ps[:], Act.Exp)
                                    nc.tensor.matmul(
                                        yps[:],
                                        va[k_i][:, 65 * h:65 * h + 65],
                                        at[:],
                                        start=(k_i == 0), stop=(k_i == nkt - 1),
                                    )
                                rc = pbs.tile([1, 512], f32, tag="rc", name="rc")
                                nc.vector.reciprocal(rc[:], yps[64:65, :])
                                s64 = pbs.tile([64, 512], f32, tag="s64",
                                               name="s64")
                                nc.gpsimd.partition_broadcast(s64[:], rc[:])
                                nc.vector.tensor_mul(
                                    ytr[pi][lo:hi, qb * 512:(qb + 1) * 512],
                                    yps[0:64, :], s64[:],
                                )

                # ---------------- Phase C: o-projection partial ----------------
                with (
                    tc.tile_pool(name="pc", bufs=3) as pc,
                    tc.tile_pool(name="pso", bufs=3, space="PSUM") as pso,
                ):
                    for nt in range(4):
                        woc = pc.tile([128, 4, 512], f32r, tag="woc", name="woc")
                        for kc in range(4):
                            nc.sync.dma_start(
                                woc[:, kc, :],
                                wo3[:, kc, nt * 512:(nt + 1) * 512].bitcast(f32r),
                            )
                        for st in range(16):
                            ps = pso.tile([128, 512], f32, tag="pso", name="ps_o")
                            for kc in range(4):
                                nc.tensor.matmul(
                                    ps[:],
                                    ytr[kc][:, st * 128:(st + 1) * 128],
                                    woc[:, kc, :],
                                    start=(kc == 0), stop=(kc == 3),
                                )
                            ostg = pc.tile([128, 512], f32, tag="ostg",
                                           name="ostg")
                            nc.scalar.copy(ostg[:], ps[:])
                            nc.sync.dma_start(
                                part[st * 128:(st + 1) * 128,
                                     nt * 512:(nt + 1) * 512],
                                ostg[:],
                            )

    nc.compile()
    return nc


def _rope_tables():
    half = DH // 2
    inv_freq = 1.0 / (10000.0 ** (np.arange(0, half, dtype=np.float32) / half))
    t = np.arange(S, dtype=np.float32)
    freqs = np.einsum("i,j->ij", t, inv_freq)            # [S, 32]
    emb = np.concatenate([freqs, freqs], axis=-1)        # [S, 64]
    cos = np.ascontiguousarray(np.cos(emb).T.astype(np.float32))  # [64, S]
    sin = np.sin(emb).T.astype(np.float32)
    rsin = np.concatenate([-sin[:32], sin[32:]], axis=0)
    return cos, np.ascontiguousarray(rsin)


def _masks():
    j = np.arange(128)[:, None, None]
    dt = np.arange(4)[None, :, None]
    i = np.arange(512)[None, None, :]
    keep = (128 * dt + j) <= i
    return np.where(keep, 0.0, -1e30).astype(np.float32)  # [128, 4, 512]


def kernel(**inputs):
    global LAST_EXEC_NS
    x = np.asarray(inputs["x"], dtype=np.float32)
    Wq = np.asarray(inputs["Wq"], dtype=np.float32)
    Wk = np.asarray(inputs["Wk"], dtype=np.float32)
    Wv = np.asarray(inputs["Wv"], dtype=np.float32)
    Wo = np.asarray(inputs["Wo"], dtype=np.float32)
    bq = np.asarray(inputs["bq"], dtype=np.float32)
    bk = np.asarray(inputs["bk"], dtype=np.float32)
    bv = np.asarray(inputs["bv"], dtype=np.float32)
    bo = np.asarray(inputs["bo"], dtype=np.float32)

    cosT, rsinT = _rope_tables()
    masks = _masks()

    nc = _build()
    in_maps = []
    for c in range(N_CORES):
        b, g = c // 4, c % 4
        sl = slice(CW * g, CW * (g + 1))
        in_maps.append({
            "xT": np.ascontiguousarray(x[b].T),
            "wq": np.ascontiguousarray(Wq[sl].T),
            "wk": np.ascontiguousarray(Wk[sl].T * 0.125),
            "wv": np.ascontiguousarray(Wv[sl].T),
            "wo": np.ascontiguousarray(Wo[:, sl].T),
            "bq": np.ascontiguousarray(bq[sl].reshape(1, CW)),
            "bk": np.ascontiguousarray((bk[sl] * 0.125).reshape(1, CW)),
            "bv": np.ascontiguousarray(bv[sl].reshape(1, CW)),
            "cosT": cosT,
            "vones": np.ones((128, 520), dtype=np.float32),
            "rsinT": rsinT,
            "masks": masks,
        })

    kwargs = {}
    if TRACE:
        kwargs = dict(trace=True, trace_cores=list(range(N_CORES)),
                      stitch_traces=False)
    r = run_bass_kernel_spmd(nc, in_maps, list(range(N_CORES)), **kwargs)
    LAST_EXEC_NS = r.exec_time_ns

    out = np.empty((B, S, D), dtype=np.float32)
    for b in range(B):
        acc = r.results[4 * b]["part"].astype(np.float32).copy()
        for g in range(1, 4):
            acc += r.results[4 * b + g]["part"]
        out[b] = acc + bo
    return out
